# revision 1
# baseline (speedup 1.0000x reference)
"""Sliding-window GQA attention (RoPE + tanh soft-cap) on 4 Trainium2 cores.

Sharding: core c = 2*b + hh with b = batch, hh = head-half. Each core handles
batch b, q-heads [8*hh, 8*hh+8), kv-heads [4*hh, 4*hh+4) as G=2 sequential
head-groups of (4 q-heads, 2 kv-heads); the host sums the 2 partials per batch.

Per group g (same inner structure as the 8-core kernel):
  A1: q^T = (q_w^T @ x^T), RoPE              -> DRAM scratch qTd slice g
  A2: k^T = (k_w^T @ x^T), RoPE              -> SBUF [512, 2048] f16
  A3: v   = (x @ v_w)                        -> SBUF [2048, 512] bf16
  B1: transposed-band attention              -> enc_sb rows [8g, 8g+8)
Then one B2: out = enc^T.T @ o_w over all 16 row-tiles -> DRAM [2048, 3584].

Wire dtype: fp16 except the exp/probs pipeline (bf16 for range).
"""

import numpy as np

B, T, D, N, KH, H = 2, 2048, 3584, 16, 8, 256
WINDOW = 1024
SOFT_CAP = 50.0
SCALAR = 0.0625
BASE = 10000.0
NEG = -30000.0  # tanh-domain mask (fits fp16; exp(50*NEG) underflows to 0)

P = 128
G = 2          # head-groups per core
NH = 4         # q heads per group
NKH = 2        # kv heads per group
KT = D // P    # 28 contraction tiles
NA = NH * (H // P)   # 8 q^T row-tiles per group
KA = NKH * (H // P)  # 4 k^T row-tiles per group
TB = T // P    # 16 query blocks
TW = 256       # query-pair width for the transposed-attention phase
NCORES = 4

_PROG_CACHE = {}


def _build_program():
    import concourse.bacc as bacc
    import concourse.tile as tile
    import concourse.mybir as mybir

    F32 = mybir.dt.float32
    F16 = mybir.dt.float16
    BF16 = mybir.dt.bfloat16
    Tanh = mybir.ActivationFunctionType.Tanh
    Exp = mybir.ActivationFunctionType.Exp

    nc = bacc.Bacc("TRN2", target_bir_lowering=False, debug=False,
                   num_devices=NCORES)

    xT = nc.dram_tensor("xT", [D, T], F16, kind="ExternalInput")
    qw = nc.dram_tensor("qw", [D, G * NH * H], F16, kind="ExternalInput")
    kw = nc.dram_tensor("kw", [D, G * NKH * H], F16, kind="ExternalInput")
    vw = nc.dram_tensor("vw", [D, G * NKH * H], F16, kind="ExternalInput")
    ow = nc.dram_tensor("ow", [G * NA * P, D], F16, kind="ExternalInput")
    cosT = nc.dram_tensor("cosT", [P, T], F16, kind="ExternalInput")
    sinT = nc.dram_tensor("sinT", [P, T], F16, kind="ExternalInput")
    maskt = nc.dram_tensor("maskt", [4 * P, TW], F16, kind="ExternalInput")
    qTd = nc.dram_tensor("qTd", [G * NA * P, T], F16, kind="Internal")
    sums_d = nc.dram_tensor("sums_d", [G * NH, T], F32, kind="Internal")
    out_p = nc.dram_tensor("out_p", [T, D], F16, kind="ExternalOutput")

    xT_v = xT.ap().rearrange("(o p) t -> p o t", p=P)      # [128, 28, 2048]
    qw_v = qw.ap().rearrange("(o p) h -> p o h", p=P)      # [128, 28, 2048]
    kw_v = kw.ap().rearrange("(o p) h -> p o h", p=P)      # [128, 28, 1024]
    vw_v = vw.ap().rearrange("(o p) h -> p o h", p=P)      # [128, 28, 1024]
    ow_v = ow.ap().rearrange("(a p) d -> p a d", p=P)      # [128, 16, 3584]
    qTd_v = qTd.ap().rearrange("(a p) t -> p a t", p=P)    # [128, 16, 2048]
    out_v = out_p.ap()

    CH = 256             # t-chunk for projections
    NCH = T // CH        # 8

    def rope_pair(vec, dst_a, dst_b, src_a, src_b, cs, sn, tmp_pool, tag):
        # dst_a = src_a*cos - src_b*sin ; dst_b = src_b*cos + src_a*sin
        t1 = tmp_pool.tile([P, CH], F32, tag=tag)
        t2 = tmp_pool.tile([P, CH], F32, tag=tag)
        vec.tensor_mul(t1, src_a, cs)
        vec.tensor_mul(t2, src_b, sn)
        vec.tensor_sub(dst_a, t1, t2)
        t3 = tmp_pool.tile([P, CH], F32, tag=tag)
        t4 = tmp_pool.tile([P, CH], F32, tag=tag)
        vec.tensor_mul(t3, src_b, cs)
        vec.tensor_mul(t4, src_a, sn)
        vec.tensor_add(dst_b, t3, t4)

    import concourse.bass as bass_mod

    with tile.TileContext(nc) as tc:
        with tc.tile_pool(name="p_tab", bufs=1) as p_tab, \
             tc.tile_pool(name="p_enc", bufs=1) as p_enc:
            cos16 = p_tab.tile([P, T], F16)
            sin16 = p_tab.tile([P, T], F16)
            nc.sync.dma_start(cos16[:], cosT.ap())
            nc.sync.dma_start(sin16[:], sinT.ap())
            cos_sb = p_tab.tile([P, T], F32)
            sin_sb = p_tab.tile([P, T], F32)
            nc.vector.tensor_copy(cos_sb[:], cos16[:])
            nc.vector.tensor_copy(sin_sb[:], sin16[:])
            mk16 = p_tab.tile([P, 4, TW], F16)
            nc.sync.dma_start(mk16[:], maskt.ap().rearrange(
                "(m p) c -> p m c", p=P))
            mk_sb = p_tab.tile([P, 4, TW], F32)
            nc.vector.tensor_copy(mk_sb[:], mk16[:])
            bias_c = p_tab.tile([P, 1], F32)
            nc.vector.memset(bias_c[:], -10.0)
            ones_sb = p_tab.tile([P, 1], BF16)
            nc.vector.memset(ones_sb[:], 1.0)

            enc_sb = p_enc.tile([P, G * NA, T], BF16)      # 64 KB/part

            for g in range(G):
                gho = g * NA        # row-tile offset of this group
                # ---------------- Phase A1: q^T -> qTd slice g ----------------
                with tc.tile_pool(name=f"p_a1_{g}", bufs=1) as pa1, \
                     tc.tile_pool(name=f"p_a1x_{g}", bufs=2) as pa1x, \
                     tc.tile_pool(name=f"p_a1r_{g}", bufs=8) as pa1r, \
                     tc.tile_pool(name=f"p_a1o_{g}", bufs=2) as pa1o, \
                     tc.tile_pool(name=f"ps_a1_{g}", bufs=2, space="PSUM") as ps_a1:
                    qw_sb = pa1.tile([P, KT, NH * H], F16, tag="qw")
                    XQ = KT // 4   # weights/x stream in 7-k-tile quarters
                    for q4 in range(4):
                        nc.sync.dma_start(
                            qw_sb[:, q4 * XQ:(q4 + 1) * XQ],
                            qw_v[:, q4 * XQ:(q4 + 1) * XQ,
                                 g * NH * H:(g + 1) * NH * H])
                    for ch in range(NCH):
                        tsl = slice(ch * CH, (ch + 1) * CH)
                        ps = ps_a1.tile([P, NA, CH], F32, tag="qps")
                        for q4 in range(4):
                            xt = pa1x.tile([P, XQ, CH], F16, tag="xt")
                            nc.sync.dma_start(
                                xt[:], xT_v[:, q4 * XQ:(q4 + 1) * XQ, tsl])
                            for dk in range(XQ):
                                k = q4 * XQ + dk
                                for j in range(NA):
                                    nc.tensor.matmul(
                                        ps[:, j],
                                        qw_sb[:, k, j * P:(j + 1) * P],
                                        xt[:, dk],
                                        start=(k == 0 and j % 2 == 0),
                                        stop=(k == KT - 1),
                                        skip_group_check=True)
                        qto = pa1o.tile([P, NA, CH], F16, tag="qto")
                        cs, sn = cos_sb[:, tsl], sin_sb[:, tsl]
                        for pr in range(NA // 2):
                            rope_pair(nc.vector, qto[:, 2 * pr],
                                      qto[:, 2 * pr + 1], ps[:, 2 * pr],
                                      ps[:, 2 * pr + 1], cs, sn, pa1r, "rtmp")
                        nc.sync.dma_start(qTd_v[:, gho:gho + NA, tsl], qto[:])

                with tc.tile_pool(name=f"p_kv_{g}", bufs=1) as p_kv:
                    kT_sb = p_kv.tile([P, KA, T], F16, tag="kT")
                    v_sb = p_kv.tile([P, TB, NKH * H], BF16, tag="v")

                    # ---------- Phase A2+A3: k^T and v, one xT pass ----------
                    with tc.tile_pool(name=f"p_a2_{g}", bufs=1) as pa2, \
                         tc.tile_pool(name=f"p_a2x_{g}", bufs=2) as pa2x, \
                         tc.tile_pool(name=f"p_a2r_{g}", bufs=8) as pa2r, \
                         tc.tile_pool(name=f"ps_a2_{g}", bufs=2,
                                      space="PSUM") as ps_a2, \
                         tc.tile_pool(name=f"ps_a3_{g}", bufs=2,
                                      space="PSUM") as ps_a3:
                        kw_sb = pa2.tile([P, KT, NKH * H], F16, tag="kw")
                        vw_sb = pa2.tile([P, KT, NKH * H], F16, tag="vw")
                        XQ = KT // 4
                        for q4 in range(4):
                            ksl = slice(q4 * XQ, (q4 + 1) * XQ)
                            hsl = slice(g * NKH * H, (g + 1) * NKH * H)
                            nc.sync.dma_start(kw_sb[:, ksl], kw_v[:, ksl, hsl])
                            nc.sync.dma_start(vw_sb[:, ksl], vw_v[:, ksl, hsl])
                        for ch in range(NCH):
                            tsl = slice(ch * CH, (ch + 1) * CH)
                            ps = ps_a2.tile([P, KA, CH], F32, tag="kps")
                            psv = ps_a3.tile([P, CH // P, NKH * H], F32,
                                             tag="vps")
                            for q4 in range(4):
                                xt = pa2x.tile([P, XQ, CH], F16, tag="xt")
                                nc.sync.dma_start(
                                    xt[:], xT_v[:, q4 * XQ:(q4 + 1) * XQ, tsl])
                                for dk in range(XQ):
                                    k = q4 * XQ + dk
                                    for j in range(KA):
                                        nc.tensor.matmul(
                                            ps[:, j],
                                            kw_sb[:, k, j * P:(j + 1) * P],
                                            xt[:, dk],
                                            start=(k == 0 and j % 2 == 0),
                                            stop=(k == KT - 1),
                                            skip_group_check=True)
                                    for st in range(CH // P):
                                        nc.tensor.matmul(
                                            psv[:, st],
                                            xt[:, dk, st * P:(st + 1) * P],
                                            vw_sb[:, k], start=(k == 0),
                                            stop=(k == KT - 1))
                            cs, sn = cos_sb[:, tsl], sin_sb[:, tsl]
                            for pr in range(KA // 2):
                                rope_pair(nc.vector, kT_sb[:, 2 * pr, tsl],
                                          kT_sb[:, 2 * pr + 1, tsl],
                                          ps[:, 2 * pr], ps[:, 2 * pr + 1],
                                          cs, sn, pa2r, "rtmp")
                            for st in range(CH // P):
                                nc.vector.tensor_copy(
                                    v_sb[:, ch * (CH // P) + st, :], psv[:, st])

                    # ---------------- Phase B1: attention -> enc_sb ----------
                    with tc.tile_pool(name=f"p_b1s_{g}", bufs=2) as pb1s, \
                         tc.tile_pool(name=f"p_b1e_{g}", bufs=3) as pb1e, \
                         tc.tile_pool(name=f"p_sr_{g}", bufs=2) as psr, \
                         tc.tile_pool(name=f"ps_lg_{g}", bufs=2,
                                      space="PSUM") as ps_lg, \
                         tc.tile_pool(name=f"ps_sm_{g}", bufs=2,
                                      space="PSUM") as ps_sm, \
                         tc.tile_pool(name=f"ps_en_{g}", bufs=2,
                                      space="PSUM") as ps_en:
                        MKJ = {0: 0, 1: 1, 8: 2, 9: 3}

                        # heads OUTER, query-pairs inner: each head's sums
                        # complete early so its normalization overlaps the
                        # next head's attention instead of stalling B2.
                        for kh in range(NKH):
                            for nl in range(2):
                                n = kh * 2 + nl
                                srow_h = psr.tile([1, T], F32, tag="srh")
                                for pr in range(T // TW):
                                    t0p = pr * TW
                                    qb = pb1s.tile([P, 2, TW], F16, tag="qb")
                                    nc.sync.dma_start(
                                        qb[:],
                                        qTd_v[:, gho + 2 * n:gho + 2 * n + 2,
                                              t0p:t0p + TW])
                                    js = max(0, 8 - 2 * pr)
                                    exps = pb1e.tile([P, 10, TW], BF16,
                                                     tag="expT")
                                    smp = ps_sm.tile([P, TW], F32, tag="smp")
                                    encp = ps_en.tile([P, 2, TW], F32, tag="en")
                                    jgroups = []
                                    j = js
                                    while j < 10:
                                        w = min(4, 10 - j)
                                        jgroups.append((j, w))
                                        j += w
                                    for gi, (j0, w) in enumerate(jgroups):
                                        lgT = ps_lg.tile([P, 4, TW], F32,
                                                         tag="lgt")
                                        for dj in range(w):
                                            j = j0 + dj
                                            s0 = (2 * pr - 8 + j) * P
                                            for hh in range(2):
                                                nc.tensor.matmul(
                                                    lgT[:, dj],
                                                    kT_sb[:, kh * 2 + hh,
                                                          s0:s0 + P],
                                                    qb[:, hh],
                                                    start=(hh == 0 and
                                                           dj % 2 == 0),
                                                    stop=(hh == 1),
                                                    skip_group_check=True)
                                        tT = pb1s.tile([P, 4, TW], F32,
                                                       tag="tT")
                                        # q_w ships unscaled; SCALAR folds in
                                        # here (logits enter only via tanh).
                                        nc.scalar.activation(
                                            tT[:, :w], lgT[:, :w], Tanh,
                                            scale=SCALAR / SOFT_CAP)
                                        for dj in range(w):
                                            j = j0 + dj
                                            if j in MKJ:
                                                nc.vector.tensor_add(
                                                    tT[:, dj], tT[:, dj],
                                                    mk_sb[:, MKJ[j]])
                                        nc.scalar.activation(
                                            exps[:, j0:j0 + w], tT[:, :w],
                                            bias=bias_c[:], func=Exp,
                                            scale=SOFT_CAP)
                                        for dj in range(w):
                                            nc.tensor.matmul(
                                                smp[0:1, :], ones_sb[:],
                                                exps[:, j0 + dj],
                                                start=(gi == 0 and dj == 0),
                                                stop=(j0 + dj == 9),
                                                skip_group_check=True)
                                        for dj in range(w):
                                            j = j0 + dj
                                            stg = 2 * pr - 8 + j
                                            for hh in range(2):
                                                nc.tensor.matmul(
                                                    encp[:, hh],
                                                    v_sb[:, stg,
                                                         kh * H + hh * P:
                                                         kh * H + (hh + 1) * P],
                                                    exps[:, j],
                                                    start=(gi == 0 and dj == 0
                                                           and hh == 0),
                                                    stop=(j == 9),
                                                    skip_group_check=True)
                                    nc.vector.tensor_copy(
                                        srow_h[:, t0p:t0p + TW], smp[0:1, :])
                                    # enc row-tile order: a = kh*4 + nl*2 + hh
                                    for hh in range(2):
                                        a = gho + 4 * kh + 2 * nl + hh
                                        nc.vector.tensor_copy(
                                            enc_sb[:, a, t0p:t0p + TW],
                                            encp[:, hh])
                                # head n's sums are complete: normalize its
                                # enc rows now, overlapping the next head's
                                # attention. The DMAs ride the scalar HWDGE
                                # ring so they never queue ahead of the sync
                                # ring's weight/x prefetches.
                                nrow = sums_d.ap()[g * NH + n:g * NH + n + 1, :]
                                nc.scalar.dma_start(nrow, srow_h[:])
                                bcast = bass_mod.AP(
                                    tensor=nrow.tensor, offset=nrow.offset,
                                    ap=[[0, P]] + [list(d) for d in nrow.ap[1:]])
                                rbc = psr.tile([P, T], F32, tag="rbc")
                                nc.scalar.dma_start(rbc[:], bcast)
                                nc.vector.reciprocal(rbc[:], rbc[:])
                                rbc_bf = psr.tile([P, T], BF16, tag="rbcb")
                                nc.vector.tensor_copy(rbc_bf[:], rbc[:])
                                for hh in range(2):
                                    a = gho + 4 * kh + 2 * nl + hh
                                    nc.vector.tensor_mul(
                                        enc_sb[:, a], enc_sb[:, a], rbc_bf[:])

            # ---------------- Phase B2: output projection (all groups) -------
            with tc.tile_pool(name="p_b2", bufs=2) as pb2, \
                 tc.tile_pool(name="p_b2o", bufs=2) as pb2o, \
                 tc.tile_pool(name="ps_b2", bufs=2, space="PSUM") as ps_b2:
                out_b = out_p.ap().rearrange("(tb p) d -> p tb d", p=P)
                for dch in range(D // 512):
                    dsl = slice(dch * 512, (dch + 1) * 512)
                    ow_sb = pb2.tile([P, G * NA, 512], BF16, tag="ow")
                    nc.gpsimd.dma_start(ow_sb[:], ow_v[:, :, dsl])
                    stage = pb2o.tile([P, TB, 512], F16, tag="ob")
                    for tb in range(TB):
                        t0 = tb * P
                        po = ps_b2.tile([P, 512], F32, tag="po")
                        for a in range(G * NA):
                            nc.tensor.matmul(
                                po[:], enc_sb[:, a, t0:t0 + P], ow_sb[:, a],
                                start=(a == 0), stop=(a == G * NA - 1))
                        nc.scalar.copy(stage[:, tb], po[:])
                    nc.sync.dma_start(out_b[:, :, dsl], stage[:])

    nc.compile()
    return nc


def _get_program():
    if "nc" not in _PROG_CACHE:
        _PROG_CACHE["nc"] = _build_program()
    return _PROG_CACHE["nc"]


def _host_inputs(x, segment_pos, q_w, kv_w, o_w):
    """Build the 4 per-core input dicts. All large operands ship as fp16."""
    BF = np.float16
    xTs = [np.ascontiguousarray(x[b].T).astype(BF) for b in range(B)]
    tabs = []
    for b in range(B):
        pos = segment_pos[b].astype(np.float64)
        inv_ts = BASE ** (-2.0 * np.arange(H // 2, dtype=np.float64) / H)
        ang = inv_ts[:, None] * pos[None, :]          # [128, T]
        tabs.append((np.cos(ang).astype(BF), np.sin(ang).astype(BF)))

    i = np.arange(P)[:, None]
    c = np.arange(TW)[None, :]
    tiles = []
    for j in (0, 1, 8, 9):
        valid = (c >= P * j + i - WINDOW) & (c <= P * j + i - 1)
        tiles.append(np.where(valid, np.float32(0.0), np.float32(NEG)))
    maskt = np.concatenate(tiles, axis=0).astype(BF)

    in_maps = []
    for core in range(NCORES):
        b, hh = divmod(core, 2)
        # group g covers q-heads [4*(2*hh+g), +4), kv-heads [2*(2*hh+g), +2)
        qg_blocks, kg_blocks, vg_blocks, ow_tiles = [], [], [], []
        for g in range(G):
            gg = 2 * hh + g
            qg_blocks.append(
                q_w[4 * gg:4 * gg + 4].transpose(1, 0, 2).reshape(D, NH * H))
            kg_blocks.append(
                kv_w[0, 2 * gg:2 * gg + 2].transpose(1, 0, 2).reshape(
                    D, NKH * H))
            vg_blocks.append(
                kv_w[1, 2 * gg:2 * gg + 2].transpose(1, 0, 2).reshape(
                    D, NKH * H))
            # row-tile order a = g*8 + kh*4 + nl*2 + hh2 matching B1 writes
            for a in range(NA):
                kh, r = divmod(a, 4)
                nl, hh2 = divmod(r, 2)
                ow_tiles.append(
                    o_w[4 * gg + 2 * kh + nl, hh2 * P:(hh2 + 1) * P, :])
        qws = np.ascontiguousarray(np.concatenate(qg_blocks, axis=1)).astype(BF)
        kws = np.ascontiguousarray(np.concatenate(kg_blocks, axis=1)).astype(BF)
        vws = np.ascontiguousarray(np.concatenate(vg_blocks, axis=1)).astype(BF)
        ows = np.ascontiguousarray(np.concatenate(ow_tiles, axis=0)).astype(BF)
        in_maps.append({
            "xT": xTs[b], "qw": qws, "kw": kws, "vw": vws, "ow": ows,
            "cosT": tabs[b][0], "sinT": tabs[b][1], "maskt": maskt,
        })
    return in_maps


def kernel(x, segment_pos, attn_mask, q_w, kv_w, o_w):
    from concourse import bass_utils

    x = np.asarray(x, dtype=np.float32)
    q_w = np.asarray(q_w, dtype=np.float32)
    kv_w = np.asarray(kv_w, dtype=np.float32)
    o_w = np.asarray(o_w, dtype=np.float32)
    segment_pos = np.asarray(segment_pos)

    nc = _get_program()
    in_maps = _host_inputs(x, segment_pos, q_w, kv_w, o_w)
    res = bass_utils.run_bass_kernel_spmd(nc, in_maps,
                                          core_ids=list(range(NCORES)))
    out = np.zeros((B, T, D), dtype=np.float32)
    for core in range(NCORES):
        out[core // 2] += res.results[core]["out_p"].astype(np.float32)
    return out



# revision 2
# speedup vs baseline: 77.0457x; 77.0457x over previous
"""Sliding-window GQA attention (RoPE + tanh soft-cap) on 8 Trainium2 cores.

Sharding: core c = 4*b + hh with b = batch, hh = head-quarter. Each core
handles batch b, q-heads [4*hh, 4*hh+4), kv-heads [2*hh, 2*hh+2) — one
head-group of (4 q-heads, 2 kv-heads); the host sums the 4 partials per batch.

Phases per core:
  A1: q^T = (q_w^T @ x^T), RoPE              -> SBUF qT_sb [128, 8, 2048] f16
  A2: k^T = (k_w^T @ x^T), RoPE              -> SBUF [512, 2048] f16
  A3: v   = (x @ v_w)                        -> SBUF [2048, 512] bf16
  B1: transposed-band attention              -> enc_sb rows [0, 8)
  B2: out = enc^T.T @ o_w over 8 row-tiles   -> DRAM [2048, 3584] partial.

Wire dtype: fp16 except the exp/probs pipeline (bf16 for range).
"""

import numpy as np

B, T, D, N, KH, H = 2, 2048, 3584, 16, 8, 256
WINDOW = 1024
SOFT_CAP = 50.0
SCALAR = 0.0625
BASE = 10000.0
NEG = -30000.0  # tanh-domain mask (fits fp16; exp(50*NEG) underflows to 0)

P = 128
NH = 4         # q heads per core
NKH = 2        # kv heads per core
KT = D // P    # 28 contraction tiles
NA = NH * (H // P)   # 8 q^T row-tiles per core
KA = NKH * (H // P)  # 4 k^T row-tiles per core
TB = T // P    # 16 query blocks
TW = 256       # query-pair width for the transposed-attention phase
NCORES = 8

_PROG_CACHE = {}


def _build_program():
    import concourse.bacc as bacc
    import concourse.tile as tile
    import concourse.mybir as mybir

    F32 = mybir.dt.float32
    F16 = mybir.dt.float16
    BF16 = mybir.dt.bfloat16
    Tanh = mybir.ActivationFunctionType.Tanh
    Exp = mybir.ActivationFunctionType.Exp

    nc = bacc.Bacc("TRN2", target_bir_lowering=False, debug=False,
                   num_devices=NCORES)

    xT = nc.dram_tensor("xT", [D, T], F16, kind="ExternalInput")
    qw = nc.dram_tensor("qw", [D, NH * H], F16, kind="ExternalInput")
    kw = nc.dram_tensor("kw", [D, NKH * H], F16, kind="ExternalInput")
    vw = nc.dram_tensor("vw", [D, NKH * H], F16, kind="ExternalInput")
    ow = nc.dram_tensor("ow", [NA * P, D], F16, kind="ExternalInput")
    cosT = nc.dram_tensor("cosT", [P, T], F16, kind="ExternalInput")
    sinT = nc.dram_tensor("sinT", [P, T], F16, kind="ExternalInput")
    maskt = nc.dram_tensor("maskt", [4 * P, TW], F16, kind="ExternalInput")
    sums_d = nc.dram_tensor("sums_d", [NH, T], F32, kind="Internal")
    out_p = nc.dram_tensor("out_p", [T, D], F16, kind="ExternalOutput")

    xT_v = xT.ap().rearrange("(o p) t -> p o t", p=P)      # [128, 28, 2048]
    qw_v = qw.ap().rearrange("(o p) h -> p o h", p=P)      # [128, 28, 1024]
    kw_v = kw.ap().rearrange("(o p) h -> p o h", p=P)      # [128, 28, 512]
    vw_v = vw.ap().rearrange("(o p) h -> p o h", p=P)      # [128, 28, 512]
    ow_v = ow.ap().rearrange("(a p) d -> p a d", p=P)      # [128, 8, 3584]

    CH = 256             # t-chunk for projections
    NCH = T // CH        # 8

    def rope_pair(vec, dst_a, dst_b, src_a, src_b, cs, sn, tmp_pool, tag):
        # dst_a = src_a*cos - src_b*sin ; dst_b = src_b*cos + src_a*sin
        t1 = tmp_pool.tile([P, CH], F32, tag=tag)
        t2 = tmp_pool.tile([P, CH], F32, tag=tag)
        vec.tensor_mul(t1, src_a, cs)
        vec.tensor_mul(t2, src_b, sn)
        vec.tensor_sub(dst_a, t1, t2)
        t3 = tmp_pool.tile([P, CH], F32, tag=tag)
        t4 = tmp_pool.tile([P, CH], F32, tag=tag)
        vec.tensor_mul(t3, src_b, cs)
        vec.tensor_mul(t4, src_a, sn)
        vec.tensor_add(dst_b, t3, t4)

    import concourse.bass as bass_mod

    with tile.TileContext(nc) as tc:
        with tc.tile_pool(name="p_tab", bufs=1) as p_tab, \
             tc.tile_pool(name="p_enc", bufs=1) as p_enc:
            cos16 = p_tab.tile([P, T], F16)
            sin16 = p_tab.tile([P, T], F16)
            nc.sync.dma_start(cos16[:], cosT.ap())
            nc.sync.dma_start(sin16[:], sinT.ap())
            cos_sb = p_tab.tile([P, T], F32)
            sin_sb = p_tab.tile([P, T], F32)
            nc.vector.tensor_copy(cos_sb[:], cos16[:])
            nc.vector.tensor_copy(sin_sb[:], sin16[:])
            mk16 = p_tab.tile([P, 4, TW], F16)
            nc.sync.dma_start(mk16[:], maskt.ap().rearrange(
                "(m p) c -> p m c", p=P))
            mk_sb = p_tab.tile([P, 4, TW], F32)
            nc.vector.tensor_copy(mk_sb[:], mk16[:])
            bias_c = p_tab.tile([P, 1], F32)
            nc.vector.memset(bias_c[:], -10.0)
            ones_sb = p_tab.tile([P, 1], BF16)
            nc.vector.memset(ones_sb[:], 1.0)

            enc_sb = p_enc.tile([P, NA, T], BF16)      # 32 KB/part
            qT_sb = p_enc.tile([P, NA, T], F16)        # 32 KB/part
            kT_sb = p_enc.tile([P, KA, T], F16)        # 16 KB/part
            v_sb = p_enc.tile([P, TB, NKH * H], BF16)  # 16 KB/part

            # ---------------- Phase A1: q^T -> qT_sb ----------------
            with tc.tile_pool(name="p_a1", bufs=1) as pa1, \
                 tc.tile_pool(name="p_a1x", bufs=2) as pa1x, \
                 tc.tile_pool(name="p_a1r", bufs=8) as pa1r, \
                 tc.tile_pool(name="ps_a1", bufs=2, space="PSUM") as ps_a1:
                qw_sb = pa1.tile([P, KT, NH * H], F16, tag="qw")
                XQ = KT // 4   # weights/x stream in 7-k-tile quarters
                for q4 in range(4):
                    nc.sync.dma_start(
                        qw_sb[:, q4 * XQ:(q4 + 1) * XQ],
                        qw_v[:, q4 * XQ:(q4 + 1) * XQ])
                for ch in range(NCH):
                    tsl = slice(ch * CH, (ch + 1) * CH)
                    ps = ps_a1.tile([P, NA, CH], F32, tag="qps")
                    for q4 in range(4):
                        xt = pa1x.tile([P, XQ, CH], F16, tag="xt")
                        nc.sync.dma_start(
                            xt[:], xT_v[:, q4 * XQ:(q4 + 1) * XQ, tsl])
                        for dk in range(XQ):
                            k = q4 * XQ + dk
                            for j in range(NA):
                                nc.tensor.matmul(
                                    ps[:, j],
                                    qw_sb[:, k, j * P:(j + 1) * P],
                                    xt[:, dk],
                                    start=(k == 0 and j % 2 == 0),
                                    stop=(k == KT - 1),
                                    skip_group_check=True)
                    cs, sn = cos_sb[:, tsl], sin_sb[:, tsl]
                    for pr in range(NA // 2):
                        rope_pair(nc.vector, qT_sb[:, 2 * pr, tsl],
                                  qT_sb[:, 2 * pr + 1, tsl], ps[:, 2 * pr],
                                  ps[:, 2 * pr + 1], cs, sn, pa1r, "rtmp")

            # ---------- Phase A2+A3: k^T and v, one xT pass ----------
            with tc.tile_pool(name="p_a2", bufs=1) as pa2, \
                 tc.tile_pool(name="p_a2x", bufs=2) as pa2x, \
                 tc.tile_pool(name="p_a2r", bufs=8) as pa2r, \
                 tc.tile_pool(name="ps_a2", bufs=2, space="PSUM") as ps_a2, \
                 tc.tile_pool(name="ps_a3", bufs=2, space="PSUM") as ps_a3:
                kw_sb = pa2.tile([P, KT, NKH * H], F16, tag="kw")
                vw_sb = pa2.tile([P, KT, NKH * H], F16, tag="vw")
                XQ = KT // 4
                for q4 in range(4):
                    ksl = slice(q4 * XQ, (q4 + 1) * XQ)
                    nc.sync.dma_start(kw_sb[:, ksl], kw_v[:, ksl])
                    nc.sync.dma_start(vw_sb[:, ksl], vw_v[:, ksl])
                for ch in range(NCH):
                    tsl = slice(ch * CH, (ch + 1) * CH)
                    ps = ps_a2.tile([P, KA, CH], F32, tag="kps")
                    psv = ps_a3.tile([P, CH // P, NKH * H], F32, tag="vps")
                    for q4 in range(4):
                        xt = pa2x.tile([P, XQ, CH], F16, tag="xt")
                        nc.sync.dma_start(
                            xt[:], xT_v[:, q4 * XQ:(q4 + 1) * XQ, tsl])
                        for dk in range(XQ):
                            k = q4 * XQ + dk
                            for j in range(KA):
                                nc.tensor.matmul(
                                    ps[:, j],
                                    kw_sb[:, k, j * P:(j + 1) * P],
                                    xt[:, dk],
                                    start=(k == 0 and j % 2 == 0),
                                    stop=(k == KT - 1),
                                    skip_group_check=True)
                            for st in range(CH // P):
                                nc.tensor.matmul(
                                    psv[:, st],
                                    xt[:, dk, st * P:(st + 1) * P],
                                    vw_sb[:, k], start=(k == 0),
                                    stop=(k == KT - 1))
                    cs, sn = cos_sb[:, tsl], sin_sb[:, tsl]
                    for pr in range(KA // 2):
                        rope_pair(nc.vector, kT_sb[:, 2 * pr, tsl],
                                  kT_sb[:, 2 * pr + 1, tsl],
                                  ps[:, 2 * pr], ps[:, 2 * pr + 1],
                                  cs, sn, pa2r, "rtmp")
                    for st in range(CH // P):
                        nc.vector.tensor_copy(
                            v_sb[:, ch * (CH // P) + st, :], psv[:, st])

            # ---------------- Phase B1: attention -> enc_sb ----------
            with tc.tile_pool(name="p_b1s", bufs=2) as pb1s, \
                 tc.tile_pool(name="p_b1e", bufs=3) as pb1e, \
                 tc.tile_pool(name="p_sr", bufs=2) as psr, \
                 tc.tile_pool(name="ps_lg", bufs=2, space="PSUM") as ps_lg, \
                 tc.tile_pool(name="ps_sm", bufs=2, space="PSUM") as ps_sm, \
                 tc.tile_pool(name="ps_en", bufs=2, space="PSUM") as ps_en:
                MKJ = {0: 0, 1: 1, 8: 2, 9: 3}

                # heads OUTER, query-pairs inner: each head's sums
                # complete early so its normalization overlaps the
                # next head's attention instead of stalling B2.
                for kh in range(NKH):
                    for nl in range(2):
                        n = kh * 2 + nl
                        srow_h = psr.tile([1, T], F32, tag="srh")
                        for pr in range(T // TW):
                            t0p = pr * TW
                            js = max(0, 8 - 2 * pr)
                            exps = pb1e.tile([P, 10, TW], BF16, tag="expT")
                            smp = ps_sm.tile([P, TW], F32, tag="smp")
                            encp = ps_en.tile([P, 2, TW], F32, tag="en")
                            jgroups = []
                            j = js
                            while j < 10:
                                w = min(4, 10 - j)
                                jgroups.append((j, w))
                                j += w
                            for gi, (j0, w) in enumerate(jgroups):
                                lgT = ps_lg.tile([P, 4, TW], F32, tag="lgt")
                                for dj in range(w):
                                    j = j0 + dj
                                    s0 = (2 * pr - 8 + j) * P
                                    for hh in range(2):
                                        nc.tensor.matmul(
                                            lgT[:, dj],
                                            kT_sb[:, kh * 2 + hh, s0:s0 + P],
                                            qT_sb[:, 2 * n + hh,
                                                  t0p:t0p + TW],
                                            start=(hh == 0 and dj % 2 == 0),
                                            stop=(hh == 1),
                                            skip_group_check=True)
                                tT = pb1s.tile([P, 4, TW], F32, tag="tT")
                                # q_w ships unscaled; SCALAR folds in
                                # here (logits enter only via tanh).
                                nc.scalar.activation(
                                    tT[:, :w], lgT[:, :w], Tanh,
                                    scale=SCALAR / SOFT_CAP)
                                for dj in range(w):
                                    j = j0 + dj
                                    if j in MKJ:
                                        nc.vector.tensor_add(
                                            tT[:, dj], tT[:, dj],
                                            mk_sb[:, MKJ[j]])
                                nc.scalar.activation(
                                    exps[:, j0:j0 + w], tT[:, :w],
                                    bias=bias_c[:], func=Exp,
                                    scale=SOFT_CAP)
                                for dj in range(w):
                                    nc.tensor.matmul(
                                        smp[0:1, :], ones_sb[:],
                                        exps[:, j0 + dj],
                                        start=(gi == 0 and dj == 0),
                                        stop=(j0 + dj == 9),
                                        skip_group_check=True)
                                for dj in range(w):
                                    j = j0 + dj
                                    stg = 2 * pr - 8 + j
                                    for hh in range(2):
                                        nc.tensor.matmul(
                                            encp[:, hh],
                                            v_sb[:, stg,
                                                 kh * H + hh * P:
                                                 kh * H + (hh + 1) * P],
                                            exps[:, j],
                                            start=(gi == 0 and dj == 0
                                                   and hh == 0),
                                            stop=(j == 9),
                                            skip_group_check=True)
                            nc.vector.tensor_copy(
                                srow_h[:, t0p:t0p + TW], smp[0:1, :])
                            # enc row-tile order: a = kh*4 + nl*2 + hh
                            for hh in range(2):
                                a = 4 * kh + 2 * nl + hh
                                nc.vector.tensor_copy(
                                    enc_sb[:, a, t0p:t0p + TW],
                                    encp[:, hh])
                        # head n's sums are complete: normalize its
                        # enc rows now, overlapping the next head's
                        # attention. The DMAs ride the scalar HWDGE
                        # ring so they never queue ahead of the sync
                        # ring's weight/x prefetches.
                        nrow = sums_d.ap()[n:n + 1, :]
                        nc.scalar.dma_start(nrow, srow_h[:])
                        bcast = bass_mod.AP(
                            tensor=nrow.tensor, offset=nrow.offset,
                            ap=[[0, P]] + [list(d) for d in nrow.ap[1:]])
                        rbc = psr.tile([P, T], F32, tag="rbc")
                        nc.scalar.dma_start(rbc[:], bcast)
                        nc.vector.reciprocal(rbc[:], rbc[:])
                        rbc_bf = psr.tile([P, T], BF16, tag="rbcb")
                        nc.vector.tensor_copy(rbc_bf[:], rbc[:])
                        for hh in range(2):
                            a = 4 * kh + 2 * nl + hh
                            nc.vector.tensor_mul(
                                enc_sb[:, a], enc_sb[:, a], rbc_bf[:])

            # ---------------- Phase B2: output projection -------------
            with tc.tile_pool(name="p_b2", bufs=2) as pb2, \
                 tc.tile_pool(name="p_b2o", bufs=2) as pb2o, \
                 tc.tile_pool(name="ps_b2", bufs=2, space="PSUM") as ps_b2:
                out_b = out_p.ap().rearrange("(tb p) d -> p tb d", p=P)
                for dch in range(D // 512):
                    dsl = slice(dch * 512, (dch + 1) * 512)
                    ow_sb = pb2.tile([P, NA, 512], BF16, tag="ow")
                    nc.gpsimd.dma_start(ow_sb[:], ow_v[:, :, dsl])
                    stage = pb2o.tile([P, TB, 512], F16, tag="ob")
                    for tb in range(TB):
                        t0 = tb * P
                        po = ps_b2.tile([P, 512], F32, tag="po")
                        for a in range(NA):
                            nc.tensor.matmul(
                                po[:], enc_sb[:, a, t0:t0 + P], ow_sb[:, a],
                                start=(a == 0), stop=(a == NA - 1))
                        nc.scalar.copy(stage[:, tb], po[:])
                    nc.sync.dma_start(out_b[:, :, dsl], stage[:])

    nc.compile()
    return nc


def _get_program():
    if "nc" not in _PROG_CACHE:
        _PROG_CACHE["nc"] = _build_program()
    return _PROG_CACHE["nc"]


def _host_inputs(x, segment_pos, q_w, kv_w, o_w):
    """Build the 8 per-core input dicts. All large operands ship as fp16."""
    BF = np.float16
    xTs = [np.ascontiguousarray(x[b].T).astype(BF) for b in range(B)]
    tabs = []
    for b in range(B):
        pos = segment_pos[b].astype(np.float64)
        inv_ts = BASE ** (-2.0 * np.arange(H // 2, dtype=np.float64) / H)
        ang = inv_ts[:, None] * pos[None, :]          # [128, T]
        tabs.append((np.cos(ang).astype(BF), np.sin(ang).astype(BF)))

    i = np.arange(P)[:, None]
    c = np.arange(TW)[None, :]
    tiles = []
    for j in (0, 1, 8, 9):
        valid = (c >= P * j + i - WINDOW) & (c <= P * j + i - 1)
        tiles.append(np.where(valid, np.float32(0.0), np.float32(NEG)))
    maskt = np.concatenate(tiles, axis=0).astype(BF)

    in_maps = []
    for core in range(NCORES):
        b, hh = divmod(core, 4)
        # q-heads [4*hh, +4), kv-heads [2*hh, +2)
        qws = np.ascontiguousarray(
            q_w[4 * hh:4 * hh + 4].transpose(1, 0, 2).reshape(
                D, NH * H)).astype(BF)
        kws = np.ascontiguousarray(
            kv_w[0, 2 * hh:2 * hh + 2].transpose(1, 0, 2).reshape(
                D, NKH * H)).astype(BF)
        vws = np.ascontiguousarray(
            kv_w[1, 2 * hh:2 * hh + 2].transpose(1, 0, 2).reshape(
                D, NKH * H)).astype(BF)
        # row-tile order a = kh*4 + nl*2 + hh2 matching B1 writes
        ow_tiles = []
        for a in range(NA):
            kh, r = divmod(a, 4)
            nl, hh2 = divmod(r, 2)
            ow_tiles.append(
                o_w[4 * hh + 2 * kh + nl, hh2 * P:(hh2 + 1) * P, :])
        ows = np.ascontiguousarray(np.concatenate(ow_tiles, axis=0)).astype(BF)
        in_maps.append({
            "xT": xTs[b], "qw": qws, "kw": kws, "vw": vws, "ow": ows,
            "cosT": tabs[b][0], "sinT": tabs[b][1], "maskt": maskt,
        })
    return in_maps


def kernel(x, segment_pos, attn_mask, q_w, kv_w, o_w):
    from concourse import bass_utils

    x = np.asarray(x, dtype=np.float32)
    q_w = np.asarray(q_w, dtype=np.float32)
    kv_w = np.asarray(kv_w, dtype=np.float32)
    o_w = np.asarray(o_w, dtype=np.float32)
    segment_pos = np.asarray(segment_pos)

    nc = _get_program()
    in_maps = _host_inputs(x, segment_pos, q_w, kv_w, o_w)
    res = bass_utils.run_bass_kernel_spmd(nc, in_maps,
                                          core_ids=list(range(NCORES)))
    out = np.zeros((B, T, D), dtype=np.float32)
    for core in range(NCORES):
        out[core // 4] += res.results[core]["out_p"].astype(np.float32)
    return out
